# revision 6
# baseline (speedup 1.0000x reference)
"""VQ codebook-lookup kernel for 8 Trainium2 NeuronCores.

Computes, for inputs [16,2048,64] and codebook [8192,64]:
  quantized = emb[argmin_k ||x - e_k||^2]  (straight-through => just the gather)
  loss      = mean((quantized - x)^2)
  indices   = argmin indices [32768] int32

Strategy (per core, 4096 tokens, data-parallel over tokens):
  - scores via PE matmul of augmented operands: s = 2*x.e - ||e||^2
    (argmax s == argmin d2).  Contraction dim C=65 (64 dims + e_norm row).
  - fp32-grade precision at 1 cycle/col via THREE fp32r matmuls
    (hi*hi + hi*res + res*hi, double-double style) accumulated in PSUM.
  - ACT drains PSUM quarters to SBUF; a custom one-pass DVE op
    (select(eq(x, running_max), Idx, 0), accum=MAX) yields the argmax
    index per token directly.
  - quantized rows gathered from DRAM by index via gpsimd indirect DMA.
  - loss partial sums on DVE; index transpose via PE for a contiguous store.
"""
import os
import sys

sys.path.insert(0, "/opt/trn_rl_repo")

import numpy as np

import concourse.bacc as bacc
import concourse.bass as bass
import concourse.dve_ops as dve_ops
import concourse.mybir as mybir
import concourse.tile as tile
from concourse.bass import IndirectOffsetOnAxis
from concourse.bass_utils import run_bass_kernel_spmd
from concourse.dve_spec import AluOp, Idx, Spec, Src0, Zero, eq, scan, select
from concourse.dve_spec import lower as dve_lower
from concourse.dve_uop import DveOpSpec

F32 = mybir.dt.float32
F32R = mybir.dt.float32r
I32 = mybir.dt.int32
OP = mybir.AluOpType
AX = mybir.AxisListType

NCORES = 8
D = 64
C = 66            # contraction: 64 dims + e_norm row + zero pad (even C for fp32r)
K = 8192          # codebook size
P = 128           # tokens per tile (partitions)
NTOK = 4096       # tokens per core
NTILE = NTOK // P  # 32
CH = 512          # codes per matmul chunk (one PSUM bank)
QW = 2048         # quarter width = 4 chunks (4 PSUM banks)
NQ = K // QW      # 4 quarters per tile
BATCH = 4         # token tiles per gather batch
NB = NTILE // BATCH  # 8 batches

_prog_cache = {}


def _round_f32r(a):
    """fp32 -> fp32r (11-bit mantissa, round-to-nearest-even), as fp32 bits."""
    bits = np.ascontiguousarray(a, dtype=np.float32).view(np.uint32)
    low = bits & np.uint32(0x00000FFF)
    rounded = bits & np.uint32(0xFFFFF000)
    add = (low > 0x800) | ((low == 0x800) & (((bits >> 12) & 1) == 1))
    rounded = rounded + (add.astype(np.uint32) << 12)
    return rounded.view(np.float32)


def _argmax_ref(in0, in1, c0, c1, c2):
    x = in0.astype(np.float32)
    r = np.maximum.accumulate(x, axis=-1)
    idx = np.arange(x.shape[-1], dtype=np.float32)
    body = np.where(x == r, idx, 0.0).astype(np.float32)
    accum = body.reshape(body.shape[0], -1).max(axis=-1, keepdims=True)
    return body, accum


def _register_argmax_op():
    """One-pass argmax along the free dim: index of the LAST running-max
    record == argmax (first occurrence under no-ties)."""
    for op in dve_ops.OPS:
        if op.name == "ARGMAX_REC":
            return op
    spec = Spec(
        body=select(eq(Src0, scan(AluOp.MAX, Src0)), Idx, Zero),
        accum=AluOp.MAX,
        reference=_argmax_ref,
    )
    shas = {}
    for ver in ("v3", "v4"):
        s = DveOpSpec(name="ARGMAX_REC", opcode=0,
                      uops=dve_lower(spec, ver=ver), rd1_en=False)
        shas[ver] = s.sha(ver)
    op = dve_ops.DveOp("ARGMAX_REC", spec, subdim=False, uops_sha=shas)
    dve_ops.OPS.append(op)
    dve_ops.CUSTOM_DVE_SPECS[op.name] = op.spec
    dve_ops._SUB_OPCODE_FOR_NAME[op.name] = (
        dve_ops._CUSTOM_DVE_ROW_BASE + len(dve_ops.OPS) - 1)
    return op


def _build_program():
    argmax_op = _register_argmax_op()
    nc = bacc.Bacc("TRN2", target_bir_lowering=False, debug=False,
                   num_devices=NCORES)

    xtr_d = nc.dram_tensor("xtr", [C, NTOK], F32R, kind="ExternalInput").ap()
    xtres_d = nc.dram_tensor("xtres", [C, NTOK], F32R, kind="ExternalInput").ap()
    etr_d = nc.dram_tensor("etr", [C, K], F32R, kind="ExternalInput").ap()
    etres_d = nc.dram_tensor("etres", [C, K], F32R, kind="ExternalInput").ap()
    xrows_d = nc.dram_tensor("xrows", [NTOK, D], F32, kind="ExternalInput").ap()
    emb_d = nc.dram_tensor("emb", [K, D], F32, kind="ExternalInput").ap()
    ident_d = nc.dram_tensor("ident", [P, P], F32, kind="ExternalInput").ap()

    quant_d = nc.dram_tensor("quant", [NTOK, D], F32, kind="ExternalOutput").ap()
    idx_d = nc.dram_tensor("idx", [NTILE, P], I32, kind="ExternalOutput").ap()
    sse_d = nc.dram_tensor("sse", [P, 1], F32, kind="ExternalOutput").ap()

    quant_re = quant_d.rearrange("(t p) d -> p t d", p=P)   # [128, 32, 64]
    xrows_re = xrows_d.rearrange("(t p) d -> p t d", p=P)   # [128, 32, 64]

    with tile.TileContext(nc) as tc:
        with tc.tile_pool(name="const", bufs=1) as cpool, \
             tc.tile_pool(name="vbuf", bufs=2) as vpool, \
             tc.tile_pool(name="small", bufs=3) as spool, \
             tc.tile_pool(name="bpool", bufs=2) as bpool:

            etr_sb = cpool.tile([C, K], F32R)
            nc.sync.dma_start(etr_sb[:], etr_d[:])
            etres_sb = cpool.tile([C, K], F32R)
            nc.sync.dma_start(etres_sb[:], etres_d[:])
            xtr_sb = cpool.tile([C, NTOK], F32R)
            nc.sync.dma_start(xtr_sb[:], xtr_d[:])
            xtres_sb = cpool.tile([C, NTOK], F32R)
            nc.sync.dma_start(xtres_sb[:], xtres_d[:])
            ident_sb = cpool.tile([P, P], F32)
            nc.sync.dma_start(ident_sb[:], ident_d[:])

            idxcol = cpool.tile([P, NTILE], F32)
            ssecols = cpool.tile([P, NB], F32)

            with tc.tile_pool(name="mm", bufs=2, space="PSUM") as mmpool:
                for t in range(NTILE):
                    xw_r = xtr_sb[:, t * P:(t + 1) * P]
                    xw_res = xtres_sb[:, t * P:(t + 1) * P]
                    V = vpool.tile([P, K], F32, tag="V")
                    for q in range(NQ):
                        ps = mmpool.tile([P, QW], F32)
                        for j in range(QW // CH):
                            ch = q * (QW // CH) + j
                            er = etr_sb[:, ch * CH:(ch + 1) * CH]
                            eres = etres_sb[:, ch * CH:(ch + 1) * CH]
                            out_sl = ps[:, j * CH:(j + 1) * CH]
                            nc.tensor.matmul(out_sl, xw_r, er,
                                             start=True, stop=False)
                            nc.tensor.matmul(out_sl, xw_r, eres,
                                             start=False, stop=False)
                            nc.tensor.matmul(out_sl, xw_res, er,
                                             start=False, stop=True)
                        nc.scalar.copy(V[:, q * QW:(q + 1) * QW], ps[:])
                    # one-pass argmax over the tile's 8192 scores (in place)
                    nc.vector._custom_dve(argmax_op, out=V[:], in0=V[:],
                                          accum_out=idxcol[:, t:t + 1])

                    if t % BATCH == BATCH - 1:
                        g = t // BATCH
                        cols = slice(g * BATCH, (g + 1) * BATCH)
                        idx32 = bpool.tile([P, BATCH], I32)
                        nc.vector.tensor_copy(idx32[:], idxcol[:, cols])
                        qg = bpool.tile([P, BATCH * D], F32)
                        qg3 = qg[:].rearrange("p (t d) -> p t d", d=D)
                        for b in range(BATCH):
                            nc.gpsimd.indirect_dma_start(
                                out=qg[:, b * D:(b + 1) * D], out_offset=None,
                                in_=emb_d[:], in_offset=IndirectOffsetOnAxis(
                                    ap=idx32[:, b:b + 1], axis=0))
                        xg = bpool.tile([P, BATCH * D], F32)
                        nc.sync.dma_start(
                            xg[:].rearrange("p (t d) -> p t d", d=D),
                            xrows_re[:, cols, :])
                        diffg = bpool.tile([P, BATCH * D], F32)
                        nc.vector.tensor_tensor(diffg[:], qg[:], xg[:],
                                                op=OP.subtract)
                        junkq = bpool.tile([P, BATCH * D], F32)
                        nc.vector.scalar_tensor_tensor(
                            junkq[:], diffg[:], 0.0, diffg[:],
                            op0=OP.bypass, op1=OP.mult,
                            accum_out=ssecols[:, g:g + 1])
                        nc.sync.dma_start(quant_re[:, cols, :], qg3)

            with tc.tile_pool(name="pst", bufs=1, space="PSUM") as pstpool:
                pst = pstpool.tile([NTILE, P], F32)
                nc.tensor.transpose(pst[:], idxcol[:], ident_sb[:])
                idxo = spool.tile([NTILE, P], I32)
                nc.vector.tensor_copy(idxo[:], pst[:])
                nc.sync.dma_start(idx_d[:], idxo[:])

            sse_t = spool.tile([P, 1], F32)
            nc.vector.tensor_reduce(sse_t[:], ssecols[:], axis=AX.X, op=OP.add)
            nc.sync.dma_start(sse_d[:], sse_t[:])

    nc.compile()
    return nc


def kernel(inputs, embeddings):
    x = np.asarray(inputs, dtype=np.float32)
    emb = np.ascontiguousarray(np.asarray(embeddings, dtype=np.float32))
    flat = x.reshape(-1, D)                       # [32768, 64]
    n_total = flat.shape[0]
    assert n_total == NCORES * NTOK and emb.shape == (K, D)

    # shared host precompute
    e2t = (2.0 * emb).T                            # [64, 8192]
    en = (emb * emb).sum(axis=1, dtype=np.float32)
    et_aug = np.zeros((C, K), np.float32)
    et_aug[:D] = e2t
    et_aug[D] = en
    etr = _round_f32r(et_aug)
    etres = _round_f32r(et_aug - etr)
    ident = np.eye(P, dtype=np.float32)

    in_maps = []
    for c in range(NCORES):
        xc = np.ascontiguousarray(flat[c * NTOK:(c + 1) * NTOK])
        xt_aug = np.zeros((C, NTOK), np.float32)
        xt_aug[:D] = xc.T
        xt_aug[D] = -1.0
        xtr = _round_f32r(xt_aug)
        xtres = _round_f32r(xt_aug - xtr)
        in_maps.append(dict(xtr=xtr, xtres=xtres, etr=etr, etres=etres,
                            xrows=xc, emb=emb, ident=ident))

    if "prog" not in _prog_cache:
        _prog_cache["prog"] = _build_program()
    nc = _prog_cache["prog"]

    trace = os.environ.get("VQ_TRACE") == "1"
    res = run_bass_kernel_spmd(nc, in_maps, list(range(NCORES)), trace=trace)
    _prog_cache["last_results"] = res

    quant = np.concatenate([res.results[c]["quant"] for c in range(NCORES)],
                           axis=0).reshape(x.shape)
    idx = np.concatenate(
        [res.results[c]["idx"].reshape(-1) for c in range(NCORES)]
    ).astype(np.int32)
    sse = np.float64(0.0)
    for c in range(NCORES):
        sse += res.results[c]["sse"].astype(np.float64).sum()
    loss = np.float32(sse / (n_total * D))
    return quant, loss, idx


# revision 7
# speedup vs baseline: 1.0050x; 1.0050x over previous
"""VQ codebook-lookup kernel for 8 Trainium2 NeuronCores.

Computes, for inputs [16,2048,64] and codebook [8192,64]:
  quantized = emb[argmin_k ||x - e_k||^2]  (straight-through => just the gather)
  loss      = mean((quantized - x)^2)
  indices   = argmin indices [32768] int32

Strategy (per core, 4096 tokens, data-parallel over tokens):
  - scores via PE matmul of augmented operands: s = 2*x.e - ||e||^2
    (argmax s == argmin d2).  Contraction dim C=65 (64 dims + e_norm row).
  - fp32-grade precision at 1 cycle/col via THREE fp32r matmuls
    (hi*hi + hi*res + res*hi, double-double style) accumulated in PSUM.
  - ACT drains PSUM quarters to SBUF; a custom one-pass DVE op
    (select(eq(x, running_max), Idx, 0), accum=MAX) yields the argmax
    index per token directly.
  - quantized rows gathered from DRAM by index via gpsimd indirect DMA.
  - loss partial sums on DVE; index transpose via PE for a contiguous store.
"""
import os
import sys

sys.path.insert(0, "/opt/trn_rl_repo")

import numpy as np

import concourse.bacc as bacc
import concourse.bass as bass
import concourse.dve_ops as dve_ops
import concourse.mybir as mybir
import concourse.tile as tile
from concourse.bass import IndirectOffsetOnAxis
from concourse.bass_utils import run_bass_kernel_spmd
from concourse.dve_spec import AluOp, Idx, Spec, Src0, Zero, eq, scan, select
from concourse.dve_spec import lower as dve_lower
from concourse.dve_uop import DveOpSpec

F32 = mybir.dt.float32
F32R = mybir.dt.float32r
I32 = mybir.dt.int32
OP = mybir.AluOpType
AX = mybir.AxisListType

NCORES = 8
D = 64
C = 66            # contraction: 64 dims + e_norm row + zero pad (even C for fp32r)
K = 8192          # codebook size
P = 128           # tokens per tile (partitions)
NTOK = 4096       # tokens per core
NTILE = NTOK // P  # 32
CH = 512          # codes per matmul chunk (one PSUM bank)
QW = 2048         # quarter width = 4 chunks (4 PSUM banks)
NQ = K // QW      # 4 quarters per tile
BATCH = 4         # token tiles per gather batch
NB = NTILE // BATCH  # 8 batches

_prog_cache = {}


def _round_f32r(a):
    """fp32 -> fp32r (11-bit mantissa, round-to-nearest-even), as fp32 bits."""
    bits = np.ascontiguousarray(a, dtype=np.float32).view(np.uint32)
    low = bits & np.uint32(0x00000FFF)
    rounded = bits & np.uint32(0xFFFFF000)
    add = (low > 0x800) | ((low == 0x800) & (((bits >> 12) & 1) == 1))
    rounded = rounded + (add.astype(np.uint32) << 12)
    return rounded.view(np.float32)


def _argmax_ref(in0, in1, c0, c1, c2):
    x = in0.astype(np.float32)
    r = np.maximum.accumulate(x, axis=-1)
    idx = np.arange(x.shape[-1], dtype=np.float32)
    body = np.where(x == r, idx, 0.0).astype(np.float32)
    accum = body.reshape(body.shape[0], -1).max(axis=-1, keepdims=True)
    return body, accum


def _register_argmax_op():
    """One-pass argmax along the free dim: index of the LAST running-max
    record == argmax (first occurrence under no-ties)."""
    for op in dve_ops.OPS:
        if op.name == "ARGMAX_REC":
            return op
    spec = Spec(
        body=select(eq(Src0, scan(AluOp.MAX, Src0)), Idx, Zero),
        accum=AluOp.MAX,
        reference=_argmax_ref,
    )
    shas = {}
    for ver in ("v3", "v4"):
        s = DveOpSpec(name="ARGMAX_REC", opcode=0,
                      uops=dve_lower(spec, ver=ver), rd1_en=False)
        shas[ver] = s.sha(ver)
    op = dve_ops.DveOp("ARGMAX_REC", spec, subdim=False, uops_sha=shas)
    dve_ops.OPS.append(op)
    dve_ops.CUSTOM_DVE_SPECS[op.name] = op.spec
    dve_ops._SUB_OPCODE_FOR_NAME[op.name] = (
        dve_ops._CUSTOM_DVE_ROW_BASE + len(dve_ops.OPS) - 1)
    return op


def _build_program():
    argmax_op = _register_argmax_op()
    nc = bacc.Bacc("TRN2", target_bir_lowering=False, debug=False,
                   num_devices=NCORES)

    xtr_d = nc.dram_tensor("xtr", [C, NTOK], F32R, kind="ExternalInput").ap()
    xtres_d = nc.dram_tensor("xtres", [C, NTOK], F32R, kind="ExternalInput").ap()
    etr_d = nc.dram_tensor("etr", [C, K], F32R, kind="ExternalInput").ap()
    etres_d = nc.dram_tensor("etres", [C, K], F32R, kind="ExternalInput").ap()
    xrows_d = nc.dram_tensor("xrows", [NTOK, D], F32, kind="ExternalInput").ap()
    emb_d = nc.dram_tensor("emb", [K, D], F32, kind="ExternalInput").ap()
    ident_d = nc.dram_tensor("ident", [P, P], F32, kind="ExternalInput").ap()

    quant_d = nc.dram_tensor("quant", [NTOK, D], F32, kind="ExternalOutput").ap()
    idx_d = nc.dram_tensor("idx", [NTILE, P], I32, kind="ExternalOutput").ap()
    sse_d = nc.dram_tensor("sse", [P, 1], F32, kind="ExternalOutput").ap()

    quant_re = quant_d.rearrange("(t p) d -> p t d", p=P)   # [128, 32, 64]
    xrows_re = xrows_d.rearrange("(t p) d -> p t d", p=P)   # [128, 32, 64]

    with tile.TileContext(nc) as tc:
        with tc.tile_pool(name="const", bufs=1) as cpool, \
             tc.tile_pool(name="vbuf", bufs=2) as vpool, \
             tc.tile_pool(name="small", bufs=3) as spool, \
             tc.tile_pool(name="bpool", bufs=2) as bpool:

            etr_sb = cpool.tile([C, K], F32R)
            nc.sync.dma_start(etr_sb[:], etr_d[:])
            etres_sb = cpool.tile([C, K], F32R)
            nc.sync.dma_start(etres_sb[:], etres_d[:])
            xtr_sb = cpool.tile([C, NTOK], F32R)
            nc.sync.dma_start(xtr_sb[:], xtr_d[:])
            xtres_sb = cpool.tile([C, NTOK], F32R)
            nc.sync.dma_start(xtres_sb[:], xtres_d[:])
            ident_sb = cpool.tile([P, P], F32)
            nc.sync.dma_start(ident_sb[:], ident_d[:])

            idxcol = cpool.tile([P, NTILE], F32)
            ssecols = cpool.tile([P, NB], F32)

            with tc.tile_pool(name="mm", bufs=2, space="PSUM") as mmpool:
                for t in range(NTILE):
                    xw_r = xtr_sb[:, t * P:(t + 1) * P]
                    xw_res = xtres_sb[:, t * P:(t + 1) * P]
                    V = vpool.tile([P, K], F32, tag="V")
                    for q in range(NQ):
                        ps = mmpool.tile([P, QW], F32)
                        # term-major order: minimize weight/moving switches
                        for w, mv_sb, st, sp in ((xw_r, etr_sb, True, False),
                                                 (xw_res, etr_sb, False, False),
                                                 (xw_r, etres_sb, False, True)):
                            for j in range(QW // CH):
                                ch = q * (QW // CH) + j
                                mv = mv_sb[:, ch * CH:(ch + 1) * CH]
                                out_sl = ps[:, j * CH:(j + 1) * CH]
                                nc.tensor.matmul(out_sl, w, mv,
                                                 start=st, stop=sp)
                        nc.scalar.copy(V[:, q * QW:(q + 1) * QW], ps[:])
                    # one-pass argmax over the tile's 8192 scores (in place)
                    nc.vector._custom_dve(argmax_op, out=V[:], in0=V[:],
                                          accum_out=idxcol[:, t:t + 1])

                    if t % BATCH == BATCH - 1:
                        g = t // BATCH
                        cols = slice(g * BATCH, (g + 1) * BATCH)
                        idx32 = bpool.tile([P, BATCH], I32)
                        nc.vector.tensor_copy(idx32[:], idxcol[:, cols])
                        qg = bpool.tile([P, BATCH * D], F32)
                        qg3 = qg[:].rearrange("p (t d) -> p t d", d=D)
                        for b in range(BATCH):
                            nc.gpsimd.indirect_dma_start(
                                out=qg[:, b * D:(b + 1) * D], out_offset=None,
                                in_=emb_d[:], in_offset=IndirectOffsetOnAxis(
                                    ap=idx32[:, b:b + 1], axis=0))
                        xg = bpool.tile([P, BATCH * D], F32)
                        nc.sync.dma_start(
                            xg[:].rearrange("p (t d) -> p t d", d=D),
                            xrows_re[:, cols, :])
                        diffg = bpool.tile([P, BATCH * D], F32)
                        nc.vector.tensor_tensor(diffg[:], qg[:], xg[:],
                                                op=OP.subtract)
                        junkq = bpool.tile([P, BATCH * D], F32)
                        nc.vector.scalar_tensor_tensor(
                            junkq[:], diffg[:], 0.0, diffg[:],
                            op0=OP.bypass, op1=OP.mult,
                            accum_out=ssecols[:, g:g + 1])
                        nc.sync.dma_start(quant_re[:, cols, :], qg3)

            with tc.tile_pool(name="pst", bufs=1, space="PSUM") as pstpool:
                pst = pstpool.tile([NTILE, P], F32)
                nc.tensor.transpose(pst[:], idxcol[:], ident_sb[:])
                idxo = spool.tile([NTILE, P], I32)
                nc.vector.tensor_copy(idxo[:], pst[:])
                nc.sync.dma_start(idx_d[:], idxo[:])

            sse_t = spool.tile([P, 1], F32)
            nc.vector.tensor_reduce(sse_t[:], ssecols[:], axis=AX.X, op=OP.add)
            nc.sync.dma_start(sse_d[:], sse_t[:])

    nc.compile()
    return nc


def kernel(inputs, embeddings):
    x = np.asarray(inputs, dtype=np.float32)
    emb = np.ascontiguousarray(np.asarray(embeddings, dtype=np.float32))
    flat = x.reshape(-1, D)                       # [32768, 64]
    n_total = flat.shape[0]
    assert n_total == NCORES * NTOK and emb.shape == (K, D)

    # shared host precompute
    e2t = (2.0 * emb).T                            # [64, 8192]
    en = (emb * emb).sum(axis=1, dtype=np.float32)
    et_aug = np.zeros((C, K), np.float32)
    et_aug[:D] = e2t
    et_aug[D] = en
    etr = _round_f32r(et_aug)
    etres = _round_f32r(et_aug - etr)
    ident = np.eye(P, dtype=np.float32)

    in_maps = []
    for c in range(NCORES):
        xc = np.ascontiguousarray(flat[c * NTOK:(c + 1) * NTOK])
        xt_aug = np.zeros((C, NTOK), np.float32)
        xt_aug[:D] = xc.T
        xt_aug[D] = -1.0
        xtr = _round_f32r(xt_aug)
        xtres = _round_f32r(xt_aug - xtr)
        in_maps.append(dict(xtr=xtr, xtres=xtres, etr=etr, etres=etres,
                            xrows=xc, emb=emb, ident=ident))

    if "prog" not in _prog_cache:
        _prog_cache["prog"] = _build_program()
    nc = _prog_cache["prog"]

    trace = os.environ.get("VQ_TRACE") == "1"
    res = run_bass_kernel_spmd(nc, in_maps, list(range(NCORES)), trace=trace)
    _prog_cache["last_results"] = res

    quant = np.concatenate([res.results[c]["quant"] for c in range(NCORES)],
                           axis=0).reshape(x.shape)
    idx = np.concatenate(
        [res.results[c]["idx"].reshape(-1) for c in range(NCORES)]
    ).astype(np.int32)
    sse = np.float64(0.0)
    for c in range(NCORES):
        sse += res.results[c]["sse"].astype(np.float64).sum()
    loss = np.float32(sse / (n_total * D))
    return quant, loss, idx


# revision 8
# speedup vs baseline: 1.7826x; 1.7738x over previous
"""VQ codebook-lookup kernel for 8 Trainium2 NeuronCores.

Computes, for inputs [16,2048,64] and codebook [8192,64]:
  quantized = emb[argmin_k ||x - e_k||^2]  (straight-through => just the gather)
  loss      = mean((quantized - x)^2)
  indices   = argmin indices [32768] int32

Strategy (per core, 4096 tokens, data-parallel over tokens):
  - scores via PE matmul of augmented operands: s = 2*x.e - ||e||^2
    (argmax s == argmin d2).  Contraction dim C=65 (64 dims + e_norm row).
  - fp32-grade precision at 1 cycle/col via THREE fp32r matmuls
    (hi*hi + hi*res + res*hi, double-double style) accumulated in PSUM.
  - ACT drains PSUM quarters to SBUF; a custom one-pass DVE op
    (select(eq(x, running_max), Idx, 0), accum=MAX) yields the argmax
    index per token directly.
  - quantized rows gathered from DRAM by index via gpsimd indirect DMA.
  - loss partial sums on DVE; index transpose via PE for a contiguous store.
"""
import os
import sys

sys.path.insert(0, "/opt/trn_rl_repo")

import numpy as np

import concourse.bacc as bacc
import concourse.bass as bass
import concourse.dve_ops as dve_ops
import concourse.mybir as mybir
import concourse.tile as tile
from concourse.bass import IndirectOffsetOnAxis
from concourse.bass_utils import run_bass_kernel_spmd
from concourse.dve_spec import AluOp, Idx, Spec, Src0, Src1, Zero, eq, scan, select
from concourse.dve_spec import lower as dve_lower
from concourse.dve_uop import DveOpSpec

F32 = mybir.dt.float32
F32R = mybir.dt.float32r
I32 = mybir.dt.int32
OP = mybir.AluOpType
AX = mybir.AxisListType

NCORES = 8
D = 64
C = 128           # contraction: [x_hi(64); x_res(64)] double-double blocks
K = 8192          # codebook size
P = 128           # tokens per tile (partitions)
NTOK = 4096       # tokens per core
NTILE = NTOK // P  # 32
CH = 512          # codes per matmul chunk (one PSUM bank)
QW = 2048         # quarter width = 4 chunks (4 PSUM banks)
NQ = K // QW      # 4 quarters per tile
BATCH = 4         # token tiles per gather batch
NB = NTILE // BATCH  # 8 batches

_prog_cache = {}


def _round_f32r(a):
    """fp32 -> fp32r (11-bit mantissa, round-to-nearest-even), as fp32 bits."""
    bits = np.ascontiguousarray(a, dtype=np.float32).view(np.uint32)
    low = bits & np.uint32(0x00000FFF)
    rounded = bits & np.uint32(0xFFFFF000)
    add = (low > 0x800) | ((low == 0x800) & (((bits >> 12) & 1) == 1))
    rounded = rounded + (add.astype(np.uint32) << 12)
    return rounded.view(np.float32)


def _argmax_ref(in0, in1, c0, c1, c2):
    x = (in0.astype(np.float32) - in1.astype(np.float32)).astype(np.float32)
    r = np.maximum.accumulate(x, axis=-1)
    idx = np.arange(x.shape[-1], dtype=np.float32)
    body = np.where(x == r, idx, 0.0).astype(np.float32)
    accum = body.reshape(body.shape[0], -1).max(axis=-1, keepdims=True)
    return body, accum


def _register_argmax_op():
    """One-pass argmax along the free dim: index of the LAST running-max
    record == argmax (first occurrence under no-ties)."""
    for op in dve_ops.OPS:
        if op.name == "ARGMAX_REC":
            return op
    d = Src0 - Src1
    spec = Spec(
        body=select(eq(d, scan(AluOp.MAX, d)), Idx, Zero),
        accum=AluOp.MAX,
        reference=_argmax_ref,
    )
    shas = {}
    for ver in ("v3", "v4"):
        s = DveOpSpec(name="ARGMAX_REC", opcode=0,
                      uops=dve_lower(spec, ver=ver), rd1_en=True)
        shas[ver] = s.sha(ver)
    op = dve_ops.DveOp("ARGMAX_REC", spec, subdim=False, uops_sha=shas)
    dve_ops.OPS.append(op)
    dve_ops.CUSTOM_DVE_SPECS[op.name] = op.spec
    dve_ops._SUB_OPCODE_FOR_NAME[op.name] = (
        dve_ops._CUSTOM_DVE_ROW_BASE + len(dve_ops.OPS) - 1)
    return op


def _build_program():
    argmax_op = _register_argmax_op()
    nc = bacc.Bacc("TRN2", target_bir_lowering=False, debug=False,
                   num_devices=NCORES)

    xt_d = nc.dram_tensor("xt", [C, NTOK], F32R, kind="ExternalInput").ap()
    et1_d = nc.dram_tensor("et1", [C, K], F32R, kind="ExternalInput").ap()
    et2_d = nc.dram_tensor("et2", [C, K], F32R, kind="ExternalInput").ap()
    enorm_d = nc.dram_tensor("enorm", [P, K], F32, kind="ExternalInput").ap()
    xrows_d = nc.dram_tensor("xrows", [NTOK, D], F32, kind="ExternalInput").ap()
    emb_d = nc.dram_tensor("emb", [K, D], F32, kind="ExternalInput").ap()
    ident_d = nc.dram_tensor("ident", [P, P], F32, kind="ExternalInput").ap()

    quant_d = nc.dram_tensor("quant", [NTOK, D], F32, kind="ExternalOutput").ap()
    idx_d = nc.dram_tensor("idx", [NTILE, P], I32, kind="ExternalOutput").ap()
    sse_d = nc.dram_tensor("sse", [P, 1], F32, kind="ExternalOutput").ap()

    quant_re = quant_d.rearrange("(t p) d -> p t d", p=P)   # [128, 32, 64]
    xrows_re = xrows_d.rearrange("(t p) d -> p t d", p=P)   # [128, 32, 64]

    with tile.TileContext(nc) as tc:
        with tc.tile_pool(name="const", bufs=1) as cpool, \
             tc.tile_pool(name="vbuf", bufs=2) as vpool, \
             tc.tile_pool(name="small", bufs=3) as spool, \
             tc.tile_pool(name="bpool", bufs=2) as bpool:

            et1_sb = cpool.tile([C, K], F32R)
            nc.sync.dma_start(et1_sb[:], et1_d[:])
            et2_sb = cpool.tile([C, K], F32R)
            nc.sync.dma_start(et2_sb[:], et2_d[:])
            xt_sb = cpool.tile([C, NTOK], F32R)
            nc.sync.dma_start(xt_sb[:], xt_d[:])
            enorm_sb = cpool.tile([P, K], F32)
            nc.sync.dma_start(enorm_sb[:], enorm_d[:])
            ident_sb = cpool.tile([P, P], F32)
            nc.sync.dma_start(ident_sb[:], ident_d[:])

            idxcol = cpool.tile([P, NTILE], F32)
            ssecols = cpool.tile([P, NB], F32)

            with tc.tile_pool(name="mm", bufs=2, space="PSUM") as mmpool:
                for t in range(NTILE):
                    xw = xt_sb[:, t * P:(t + 1) * P]
                    V = vpool.tile([P, K], F32, tag="V")
                    for q in range(NQ):
                        ps = mmpool.tile([P, QW], F32)
                        for mv_sb, st, sp in ((et1_sb, True, False),
                                              (et2_sb, False, True)):
                            for j in range(QW // CH):
                                ch = q * (QW // CH) + j
                                mv = mv_sb[:, ch * CH:(ch + 1) * CH]
                                out_sl = ps[:, j * CH:(j + 1) * CH]
                                nc.tensor.matmul(out_sl, xw, mv,
                                                 start=st, stop=sp)
                        nc.scalar.copy(V[:, q * QW:(q + 1) * QW], ps[:])
                    # one-pass argmax of (V - e_norm) over the tile (in place)
                    nc.vector._custom_dve(argmax_op, out=V[:], in0=V[:],
                                          in1=enorm_sb[:],
                                          accum_out=idxcol[:, t:t + 1])

                    if t % BATCH == BATCH - 1:
                        g = t // BATCH
                        cols = slice(g * BATCH, (g + 1) * BATCH)
                        idx32 = bpool.tile([P, BATCH], I32)
                        nc.vector.tensor_copy(idx32[:], idxcol[:, cols])
                        qg = bpool.tile([P, BATCH * D], F32)
                        qg3 = qg[:].rearrange("p (t d) -> p t d", d=D)
                        for b in range(BATCH):
                            nc.gpsimd.indirect_dma_start(
                                out=qg[:, b * D:(b + 1) * D], out_offset=None,
                                in_=emb_d[:], in_offset=IndirectOffsetOnAxis(
                                    ap=idx32[:, b:b + 1], axis=0))
                        xg = bpool.tile([P, BATCH * D], F32)
                        nc.sync.dma_start(
                            xg[:].rearrange("p (t d) -> p t d", d=D),
                            xrows_re[:, cols, :])
                        diffg = bpool.tile([P, BATCH * D], F32)
                        nc.vector.tensor_tensor(diffg[:], qg[:], xg[:],
                                                op=OP.subtract)
                        junkq = bpool.tile([P, BATCH * D], F32)
                        nc.vector.scalar_tensor_tensor(
                            junkq[:], diffg[:], 0.0, diffg[:],
                            op0=OP.bypass, op1=OP.mult,
                            accum_out=ssecols[:, g:g + 1])
                        nc.sync.dma_start(quant_re[:, cols, :], qg3)

            with tc.tile_pool(name="pst", bufs=1, space="PSUM") as pstpool:
                pst = pstpool.tile([NTILE, P], F32)
                nc.tensor.transpose(pst[:], idxcol[:], ident_sb[:])
                idxo = spool.tile([NTILE, P], I32)
                nc.vector.tensor_copy(idxo[:], pst[:])
                nc.sync.dma_start(idx_d[:], idxo[:])

            sse_t = spool.tile([P, 1], F32)
            nc.vector.tensor_reduce(sse_t[:], ssecols[:], axis=AX.X, op=OP.add)
            nc.sync.dma_start(sse_d[:], sse_t[:])

    nc.compile()
    return nc


def kernel(inputs, embeddings):
    x = np.asarray(inputs, dtype=np.float32)
    emb = np.ascontiguousarray(np.asarray(embeddings, dtype=np.float32))
    flat = x.reshape(-1, D)                       # [32768, 64]
    n_total = flat.shape[0]
    assert n_total == NCORES * NTOK and emb.shape == (K, D)

    # shared host precompute
    e2t = (2.0 * emb).T.astype(np.float32)         # [64, 8192]
    en = (emb * emb).sum(axis=1, dtype=np.float32)
    er = _round_f32r(e2t)
    es = _round_f32r(e2t - er)
    et1 = np.concatenate([er, es], axis=0)         # [128, 8192]
    et2 = np.concatenate([es, er], axis=0)
    enorm = np.ascontiguousarray(
        np.broadcast_to(en[None, :], (P, K)).astype(np.float32))
    ident = np.eye(P, dtype=np.float32)

    in_maps = []
    for c in range(NCORES):
        xc = np.ascontiguousarray(flat[c * NTOK:(c + 1) * NTOK])
        xT = np.ascontiguousarray(xc.T)
        xr = _round_f32r(xT)
        xs = _round_f32r(xT - xr)
        xt = np.concatenate([xr, xs], axis=0)      # [128, 4096]
        in_maps.append(dict(xt=xt, et1=et1, et2=et2, enorm=enorm,
                            xrows=xc, emb=emb, ident=ident))

    if "prog" not in _prog_cache:
        _prog_cache["prog"] = _build_program()
    nc = _prog_cache["prog"]

    trace = os.environ.get("VQ_TRACE") == "1"
    res = run_bass_kernel_spmd(nc, in_maps, list(range(NCORES)), trace=trace)
    _prog_cache["last_results"] = res

    quant = np.concatenate([res.results[c]["quant"] for c in range(NCORES)],
                           axis=0).reshape(x.shape)
    idx = np.concatenate(
        [res.results[c]["idx"].reshape(-1) for c in range(NCORES)]
    ).astype(np.int32)
    sse = np.float64(0.0)
    for c in range(NCORES):
        sse += res.results[c]["sse"].astype(np.float64).sum()
    loss = np.float32(sse / (n_total * D))
    return quant, loss, idx


# revision 10
# speedup vs baseline: 1.8328x; 1.0281x over previous
"""VQ codebook-lookup kernel for 8 Trainium2 NeuronCores.

Computes, for inputs [16,2048,64] and codebook [8192,64]:
  quantized = emb[argmin_k ||x - e_k||^2]  (straight-through => just the gather)
  loss      = mean((quantized - x)^2)
  indices   = argmin indices [32768] int32

Strategy (per core, 4096 tokens, data-parallel over tokens):
  - scores s = 2*x.e computed on the PE at fp32-grade precision via TWO
    C=128 fp32r block matmuls (double-double: [x_hi;x_res].[e_hi;e_res]
    plus the swapped-halves variant = all four cross terms), accumulated
    in PSUM at 1 cycle/col.
  - ACT drains PSUM quarters to SBUF; a custom one-pass DVE op
    (d = s - ||e||^2; select(eq(d, running_max(d)), Idx, 0), accum=MAX)
    yields argmax(2x.e - ||e||^2) == argmin ||x-e||^2 per token directly.
  - quantized rows gathered from DRAM by index via gpsimd indirect DMA.
  - loss partial sums on DVE; index transpose via PE for a contiguous store.
"""
import os
import sys

sys.path.insert(0, "/opt/trn_rl_repo")

import numpy as np

import concourse.bacc as bacc
import concourse.bass as bass
import concourse.dve_ops as dve_ops
import concourse.mybir as mybir
import concourse.tile as tile
from concourse.bass import IndirectOffsetOnAxis
from concourse.bass_utils import run_bass_kernel_spmd
from concourse.dve_spec import AluOp, Idx, Spec, Src0, Src1, Zero, eq, scan, select
from concourse.dve_spec import lower as dve_lower
from concourse.dve_uop import DveOpSpec

F32 = mybir.dt.float32
F32R = mybir.dt.float32r
I32 = mybir.dt.int32
OP = mybir.AluOpType
AX = mybir.AxisListType

NCORES = 8
D = 64
C = 128           # contraction: [x_hi(64); x_res(64)] double-double blocks
K = 8192          # codebook size
P = 128           # tokens per tile (partitions)
NTOK = 4096       # tokens per core
NTILE = NTOK // P  # 32
CH = 512          # codes per matmul chunk (one PSUM bank)
QW = 1024         # drain-block width = 2 chunks (2 PSUM banks)
NQ = K // QW      # 4 quarters per tile
BATCH = 4         # token tiles per gather batch
NB = NTILE // BATCH  # 8 batches

_prog_cache = {}


def _round_f32r(a):
    """fp32 -> fp32r (11-bit mantissa, round-to-nearest-even), as fp32 bits."""
    bits = np.ascontiguousarray(a, dtype=np.float32).view(np.uint32)
    low = bits & np.uint32(0x00000FFF)
    rounded = bits & np.uint32(0xFFFFF000)
    add = (low > 0x800) | ((low == 0x800) & (((bits >> 12) & 1) == 1))
    rounded = rounded + (add.astype(np.uint32) << 12)
    return rounded.view(np.float32)


def _argmax_ref(in0, in1, c0, c1, c2):
    x = (in0.astype(np.float32) - in1.astype(np.float32)).astype(np.float32)
    r = np.maximum.accumulate(x, axis=-1)
    idx = np.arange(x.shape[-1], dtype=np.float32)
    body = np.where(x == r, idx, 0.0).astype(np.float32)
    accum = body.reshape(body.shape[0], -1).max(axis=-1, keepdims=True)
    return body, accum


def _register_argmax_op():
    """One-pass argmax along the free dim: index of the LAST running-max
    record == argmax (first occurrence under no-ties)."""
    for op in dve_ops.OPS:
        if op.name == "ARGMAX_REC":
            return op
    d = Src0 - Src1
    spec = Spec(
        body=select(eq(d, scan(AluOp.MAX, d)), Idx, Zero),
        accum=AluOp.MAX,
        reference=_argmax_ref,
    )
    shas = {}
    for ver in ("v3", "v4"):
        s = DveOpSpec(name="ARGMAX_REC", opcode=0,
                      uops=dve_lower(spec, ver=ver), rd1_en=True)
        shas[ver] = s.sha(ver)
    op = dve_ops.DveOp("ARGMAX_REC", spec, subdim=False, uops_sha=shas)
    dve_ops.OPS.append(op)
    dve_ops.CUSTOM_DVE_SPECS[op.name] = op.spec
    dve_ops._SUB_OPCODE_FOR_NAME[op.name] = (
        dve_ops._CUSTOM_DVE_ROW_BASE + len(dve_ops.OPS) - 1)
    return op


def _build_program():
    argmax_op = _register_argmax_op()
    nc = bacc.Bacc("TRN2", target_bir_lowering=False, debug=False,
                   num_devices=NCORES)

    xt_d = nc.dram_tensor("xt", [C, NTOK], F32R, kind="ExternalInput").ap()
    et1_d = nc.dram_tensor("et1", [C, K], F32R, kind="ExternalInput").ap()
    et2_d = nc.dram_tensor("et2", [C, K], F32R, kind="ExternalInput").ap()
    enorm_d = nc.dram_tensor("enorm", [P, K], F32, kind="ExternalInput").ap()
    xrows_d = nc.dram_tensor("xrows", [NTOK, D], F32, kind="ExternalInput").ap()
    emb_d = nc.dram_tensor("emb", [K, D], F32, kind="ExternalInput").ap()
    ident_d = nc.dram_tensor("ident", [P, P], F32, kind="ExternalInput").ap()

    quant_d = nc.dram_tensor("quant", [NTOK, D], F32, kind="ExternalOutput").ap()
    idx_d = nc.dram_tensor("idx", [NTILE, P], I32, kind="ExternalOutput").ap()
    sse_d = nc.dram_tensor("sse", [P, 1], F32, kind="ExternalOutput").ap()

    quant_re = quant_d.rearrange("(t p) d -> p t d", p=P)   # [128, 32, 64]
    xrows_re = xrows_d.rearrange("(t p) d -> p t d", p=P)   # [128, 32, 64]

    with tile.TileContext(nc) as tc:
        with tc.tile_pool(name="const", bufs=1) as cpool, \
             tc.tile_pool(name="vbuf", bufs=2) as vpool, \
             tc.tile_pool(name="small", bufs=3) as spool, \
             tc.tile_pool(name="bpool", bufs=2) as bpool:

            et1_sb = cpool.tile([C, K], F32R)
            nc.sync.dma_start(et1_sb[:], et1_d[:])
            et2_sb = cpool.tile([C, K], F32R)
            nc.sync.dma_start(et2_sb[:], et2_d[:])
            xt_sb = cpool.tile([C, NTOK], F32R)
            nc.sync.dma_start(xt_sb[:], xt_d[:])
            enorm_sb = cpool.tile([P, K], F32)
            nc.sync.dma_start(enorm_sb[:], enorm_d[:])
            ident_sb = cpool.tile([P, P], F32)
            nc.sync.dma_start(ident_sb[:], ident_d[:])

            idxcol = cpool.tile([P, NTILE], F32)
            ssecols = cpool.tile([P, NB], F32)

            with tc.tile_pool(name="mm", bufs=4, space="PSUM") as mmpool:
                for t in range(NTILE):
                    xw = xt_sb[:, t * P:(t + 1) * P]
                    V = vpool.tile([P, K], F32, tag="V")
                    for q in range(NQ):
                        ps = mmpool.tile([P, QW], F32)
                        for mv_sb, st, sp in ((et1_sb, True, False),
                                              (et2_sb, False, True)):
                            for j in range(QW // CH):
                                ch = q * (QW // CH) + j
                                mv = mv_sb[:, ch * CH:(ch + 1) * CH]
                                out_sl = ps[:, j * CH:(j + 1) * CH]
                                nc.tensor.matmul(out_sl, xw, mv,
                                                 start=st, stop=sp)
                        nc.scalar.copy(V[:, q * QW:(q + 1) * QW], ps[:])
                    # one-pass argmax of (V - e_norm) over the tile (in place)
                    nc.vector._custom_dve(argmax_op, out=V[:], in0=V[:],
                                          in1=enorm_sb[:],
                                          accum_out=idxcol[:, t:t + 1])

                    if t % BATCH == BATCH - 1:
                        g = t // BATCH
                        cols = slice(g * BATCH, (g + 1) * BATCH)
                        idx32 = bpool.tile([P, BATCH], I32)
                        nc.vector.tensor_copy(idx32[:], idxcol[:, cols])
                        qg = bpool.tile([P, BATCH * D], F32)
                        qg3 = qg[:].rearrange("p (t d) -> p t d", d=D)
                        for b in range(BATCH):
                            nc.gpsimd.indirect_dma_start(
                                out=qg[:, b * D:(b + 1) * D], out_offset=None,
                                in_=emb_d[:], in_offset=IndirectOffsetOnAxis(
                                    ap=idx32[:, b:b + 1], axis=0))
                        xg = bpool.tile([P, BATCH * D], F32)
                        nc.sync.dma_start(
                            xg[:].rearrange("p (t d) -> p t d", d=D),
                            xrows_re[:, cols, :])
                        diffg = bpool.tile([P, BATCH * D], F32)
                        nc.vector.tensor_tensor(diffg[:], qg[:], xg[:],
                                                op=OP.subtract)
                        junkq = bpool.tile([P, BATCH * D], F32)
                        nc.vector.scalar_tensor_tensor(
                            junkq[:], diffg[:], 0.0, diffg[:],
                            op0=OP.bypass, op1=OP.mult,
                            accum_out=ssecols[:, g:g + 1])
                        nc.sync.dma_start(quant_re[:, cols, :], qg3)

            with tc.tile_pool(name="pst", bufs=1, space="PSUM") as pstpool:
                pst = pstpool.tile([NTILE, P], F32)
                nc.tensor.transpose(pst[:], idxcol[:], ident_sb[:])
                idxo = spool.tile([NTILE, P], I32)
                nc.vector.tensor_copy(idxo[:], pst[:])
                nc.sync.dma_start(idx_d[:], idxo[:])

            sse_t = spool.tile([P, 1], F32)
            nc.vector.tensor_reduce(sse_t[:], ssecols[:], axis=AX.X, op=OP.add)
            nc.sync.dma_start(sse_d[:], sse_t[:])

    nc.compile()
    return nc


def kernel(inputs, embeddings):
    x = np.asarray(inputs, dtype=np.float32)
    emb = np.ascontiguousarray(np.asarray(embeddings, dtype=np.float32))
    flat = x.reshape(-1, D)                       # [32768, 64]
    n_total = flat.shape[0]
    assert n_total == NCORES * NTOK and emb.shape == (K, D)

    # shared host precompute
    e2t = (2.0 * emb).T.astype(np.float32)         # [64, 8192]
    en = (emb * emb).sum(axis=1, dtype=np.float32)
    er = _round_f32r(e2t)
    es = _round_f32r(e2t - er)
    et1 = np.concatenate([er, es], axis=0)         # [128, 8192]
    et2 = np.concatenate([es, er], axis=0)
    enorm = np.ascontiguousarray(
        np.broadcast_to(en[None, :], (P, K)).astype(np.float32))
    ident = np.eye(P, dtype=np.float32)

    in_maps = []
    for c in range(NCORES):
        xc = np.ascontiguousarray(flat[c * NTOK:(c + 1) * NTOK])
        xT = np.ascontiguousarray(xc.T)
        xr = _round_f32r(xT)
        xs = _round_f32r(xT - xr)
        xt = np.concatenate([xr, xs], axis=0)      # [128, 4096]
        in_maps.append(dict(xt=xt, et1=et1, et2=et2, enorm=enorm,
                            xrows=xc, emb=emb, ident=ident))

    if "prog" not in _prog_cache:
        _prog_cache["prog"] = _build_program()
    nc = _prog_cache["prog"]

    trace = os.environ.get("VQ_TRACE") == "1"
    res = run_bass_kernel_spmd(nc, in_maps, list(range(NCORES)), trace=trace)
    _prog_cache["last_results"] = res

    quant = np.concatenate([res.results[c]["quant"] for c in range(NCORES)],
                           axis=0).reshape(x.shape)
    idx = np.concatenate(
        [res.results[c]["idx"].reshape(-1) for c in range(NCORES)]
    ).astype(np.int32)
    sse = np.float64(0.0)
    for c in range(NCORES):
        sse += res.results[c]["sse"].astype(np.float64).sum()
    loss = np.float32(sse / (n_total * D))
    return quant, loss, idx
